# revision 44
# baseline (speedup 1.0000x reference)
"""Trainium2 Bass kernel for CrossModalAttention (attention + residual + LayerNorm).

Math: the reference concatenates [query, key_value], projects Q/K/V, attends with an
additive -10000 mask on key columns < N_q, and keeps only the query-row outputs.
exp(-10000 + s) == 0 in fp32, so this is exactly standard cross-attention:
    Q = query @ Wq + bq ; K = key_value @ Wk (+bk) ; V = key_value @ Wv + bv
    C = softmax(Q K^T / 8) V ;  out = LayerNorm(query + C) * gamma + beta
bk shifts every score of a query row equally (score += Q_q . bk for all keys),
so softmax cancels it -> bk is dropped exactly. bv is constant over keys, so
ctx = PV/denom + bv -> folded into the residual (xqb = query + bv) host-side.

Sharding: 8 cores = 2 batches x 4 query-blocks of 512 rows. Each core computes
K/V projections for its batch (duplicated across the 4 cores of a batch) and
full 8-head attention + LN for its query block.

Per-core pipeline (v3):
  - Q/K projections in bf16 (fp8 upstream puts max-rel-err at the 2e-2 gate;
    fp8 K alone measures 2.9e-2), outputs stored e4m3. V path fully fp8-DR.
  - Head h's 64 dims land naturally on partitions 64*(h%2)..+64 of chunk
    h//2 (no permutation). Scores run plain fp8 (not DoubleRow: the 256-col
    DR stationary load is slower than the matmul it feeds): [64,128]
    stationaries get FWL-fast LDWEIGHTS and the two heads of a pair occupy
    disjoint row halves -> concurrent matmuls, full-array utilization.
  - probs e5m2 everywhere (e4m3 max 240 overflows to inf on observed
    scores): even kc -> ScalarE ACT exp, odd kc -> VectorE Schraudolph
    affine bit-cast through uint8 (saturates negatives to +0; no NaN
    cliffs). Per-kc engine split makes the two exps of each psc pair run
    concurrently; a few units send both halves to ACT for balance.
  - PV in fp8 DR over ones-augmented V (row 64 = softmax denominator);
    PE-transpose + reciprocal + broadcast-stride multiply normalize; the
    final head pair is processed per-q-chunk so LayerNorm and output DMA
    overlap the finish.
  - 3-deep [128,1024] PSUM scoring pool (6 banks) + 2 PV banks; DMAs are
    batched into 7 tensors across 3 rings, m-major weight blocks so each
    projection step waits only on its own slice.
"""

import os
import sys

import numpy as np

try:
    import concourse.bass as bass  # noqa: F401
except ImportError:
    for _p in ("/opt/trn_rl_repo", "/root/.axon_site/_ro/trn_rl_repo"):
        if os.path.isdir(_p):
            sys.path.insert(0, _p)
            break
    import concourse.bass as bass  # noqa: F401

import ml_dtypes
import concourse.tile as tile
from concourse import bacc, bass_utils, mybir
from concourse.masks import make_identity

F32 = mybir.dt.float32
F16 = mybir.dt.float16
BF16 = mybir.dt.bfloat16
F8 = mybir.dt.float8e4
F8E5 = mybir.dt.float8e5
U8 = mybir.dt.uint8
AF = mybir.ActivationFunctionType
ALU = mybir.AluOpType
DR = mybir.MatmulPerfMode.DoubleRow
NP8 = mybir.dt.np(F8)
NPBF = ml_dtypes.bfloat16

B, N_Q, N_KV, D, H, HD = 2, 2048, 2048, 512, 8, 64
N_CORES = 8
QBLK = N_Q // 4          # 512 query rows per core
DC = D // 128            # 4 partition chunks of the model dim
CP = DC // 2             # 2 chunk-pairs (fp8 DoubleRow)
KC = N_KV // 128         # 16 key chunks
KCP = KC // 2            # 8 key chunk-pairs
HP = H // 2              # 4 head pairs
LN_EPS = 1e-5
SM_SCALE = 0.125
EXP_BIAS = -2.0          # probs = exp(s/8 - 2); e5m2 has no reachable cliff

# Schraudolph constants for e5m2 output via uint8: u8 = A8*s_raw + B8
A8 = 4.0 * np.log2(np.e) * SM_SCALE
B8 = 60.0 + 4.0 * np.log2(np.e) * EXP_BIAS - 4.0 * np.log2(1.0301)

# exp engine per (unit, kc): even kc -> ScalarE ACT exp, odd kc -> VectorE
# Schraudolph, so the two exps of a unit run concurrently on both engines.
# A few units send both halves to ACT to balance total throughput (36/28).
BOTH_ACT_UNITS = {11, 27}


def _build_program():
    nc = bacc.Bacc("TRN2", target_bir_lowering=False, debug=False,
                   num_devices=N_CORES)

    def din(name, shape, dt):
        return nc.dram_tensor(name, shape, dt, kind="ExternalInput").ap()

    # batched input tensors (few, large DMAs)
    ta = din("ta", [128, 4096], BF16)    # xqt c0..3 (2048) | wq c0..3 (2048)
    ta2 = din("ta2", [128, 2048], BF16)  # wk c0..3
    tb = [din(f"tb{h}", [128, 4096], BF16) for h in range(2)]  # kvT [c(4) x 1024keys]
    tc_ = din("tc", [128, 10240], F8)    # kvt8 key-quarters (8192) | wv8 (2048)
    td = din("td", [128, 3072], F16)     # xqb (2048) | gamma (512) | beta (512)
    te = din("te", [128, 4], F32)        # bq chunked
    out = nc.dram_tensor("out", [QBLK, D], F16, kind="ExternalOutput").ap()

    with tile.TileContext(nc) as tc:
        with (
            tc.tile_pool(name="persist", bufs=1) as pp,
            tc.tile_pool(name="work", bufs=4) as wkp,
            tc.tile_pool(name="small", bufs=8) as smp,
            tc.tile_pool(name="sc_ps", bufs=3, space="PSUM") as scps,
            tc.tile_pool(name="pv_ps", bufs=1, space="PSUM") as pvps,
        ):
            # ---- persistent tiles + loads ----
            a_sb = pp.tile([128, 4096], BF16, name="ta", tag="ta")
            a2_sb = pp.tile([128, 2048], BF16, name="ta2", tag="ta2")
            b_sb = [pp.tile([128, 4096], BF16, name=f"tb{h}", tag=f"tb{h}")
                    for h in range(2)]
            c_sb = pp.tile([128, 10240], F8, name="tc", tag="tc")
            d_sb = pp.tile([128, 3072], F16, name="td", tag="td")
            e_sb = pp.tile([128, 4], F32, name="te", tag="te")

            # four DMA rings; W blocks are m-major and the fp8 tensor is
            # key-quarter-major so every proj step waits only on its slice.
            nc.sync.dma_start(e_sb[:], te)
            nc.sync.dma_start(a_sb[:, 0:2560], ta[:, 0:2560])     # xqt + wq m0
            nc.sync.dma_start(a2_sb[:, 0:512], ta2[:, 0:512])     # wk m0
            nc.sync.dma_start(b_sb[0][:, 0:2048], tb[0][:, 0:2048])
            nc.sync.dma_start(a_sb[:, 2560:4096], ta[:, 2560:4096])
            nc.sync.dma_start(a2_sb[:, 512:1024], ta2[:, 512:1024])
            nc.gpsimd.dma_start(c_sb[:, 8192:10240], tc_[:, 8192:10240])  # wv
            nc.gpsimd.dma_start(c_sb[:, 0:2048], tc_[:, 0:2048])          # keys q0
            nc.gpsimd.dma_start(c_sb[:, 2048:4096], tc_[:, 2048:4096])    # keys q1
            nc.gpsimd.dma_start(c_sb[:, 4096:8192], tc_[:, 4096:8192])    # keys q2,q3
            nc.scalar.dma_start(b_sb[0][:, 2048:4096], tb[0][:, 2048:4096])
            nc.scalar.dma_start(d_sb[:], td)
            nc.scalar.dma_start(b_sb[1][:], tb[1])
            nc.scalar.dma_start(a2_sb[:, 1024:2048], ta2[:, 1024:2048])

            def xqt_c(c):
                return a_sb[:, c * 512:(c + 1) * 512]

            def wq_cm(c, m):
                return a_sb[:, 2048 + m * 512 + c * 128: 2048 + m * 512 + (c + 1) * 128]

            def wk_cm(c, m):
                return a2_sb[:, m * 512 + c * 128: m * 512 + (c + 1) * 128]

            def kvt8_t(cp, t):
                # key-quarter kq = t//4 block: [kq][cp][i][512 keys]
                kq, tr = t // 4, t % 4
                base = kq * 2048 + cp * 1024
                return c_sb[:, base:base + 1024].rearrange(
                    "p (i n) -> p i n", i=2)[:, :, tr * 128:(tr + 1) * 128]

            def wv_v(cp):
                return c_sb[:, 8192 + cp * 1024: 8192 + (cp + 1) * 1024].rearrange(
                    "p (i n) -> p i n", i=2)

            xqb = d_sb[:, 0:2048]
            gamma = d_sb[:, 2048:2560]
            beta = d_sb[:, 2560:3072]

            # HAM warmup: ~7us of back-to-back dummy matmuls during the
            # input-DMA shadow gets the PE to 2.4GHz before the real
            # projections start (otherwise they all run at the cold 1.2GHz).
            wrm = pp.tile([128, 512], BF16, name="wrm", tag="wrm")
            nc.vector.memset(wrm[:], 0.125)

            def warm_burst(n):
                for _i in range(n):
                    psd = scps.tile([128, 512], F32, name="psd", tag="sc")
                    nc.tensor.matmul(psd[:], wrm[:, 0:128], wrm[:],
                                     start=True, stop=True)
            warm_burst(28)

            ident = pp.tile([128, 128], F32, name="ident", tag="ident")
            make_identity(nc, ident[:])
            ebias = pp.tile([128, 1], F32, name="ebias", tag="ebias")
            nc.vector.memset(ebias[:], float(EXP_BIAS))
            eps_sb = pp.tile([128, 1], F32, name="eps", tag="eps")
            nc.vector.memset(eps_sb[:], float(LN_EPS))

            qt_sb = [pp.tile([128, 1024], F8, name=f"qt{g}", tag=f"qt{g}") for g in range(2)]
            kt_sb = [pp.tile([128, 4096], F8, name=f"kt{g}", tag=f"kt{g}") for g in range(2)]
            vaug_sb = [pp.tile([128, 2048], F8, name=f"va{t}", tag=f"va{t}")
                       for t in range(KCP)]
            cbig = pp.tile([128, 2048], F32, name="cbig", tag="cbig")

            def proj_qt(m):
                ps = scps.tile([128, 512], F32, name="ps_qt", tag="sc")
                for c in range(DC):
                    nc.tensor.matmul(ps[:], wq_cm(c, m), xqt_c(c),
                                     start=(c == 0), stop=(c == DC - 1))
                nc.scalar.add(
                    qt_sb[m // 2][:, (m % 2) * 512:(m % 2 + 1) * 512],
                    ps[:], e_sb[:, m:m + 1])

            def proj_kt(m, nbb):
                # keys nbb*1024 .. +1024 of output-dim chunk m (bf16, no bias)
                ps = scps.tile([128, 1024], F32, name="ps_kt", tag="sc")
                for nn in range(2):
                    for c in range(DC):
                        mov = b_sb[nbb][:, c * 1024 + nn * 512: c * 1024 + (nn + 1) * 512]
                        nc.tensor.matmul(ps[:, nn * 512:(nn + 1) * 512],
                                         wk_cm(c, m), mov,
                                         start=(c == 0), stop=(c == DC - 1))
                dst = kt_sb[m // 2][:, (m % 2) * 2048 + nbb * 1024:(m % 2) * 2048 + (nbb + 1) * 1024]
                if m < 2:
                    nc.vector.tensor_copy(dst, ps[:])
                else:
                    nc.scalar.copy(dst, ps[:])

            _vps = {}

            def proj_v(t):
                # pair t,t+1 into one [128,1024] psum; one copy per pair
                if t % 2 == 0:
                    _vps[t // 2] = scps.tile([128, 1024], F32, name="ps_v", tag="sc")
                ps = _vps[t // 2]
                half = ps[:, (t % 2) * 512:(t % 2 + 1) * 512]
                for cp in range(CP):
                    nc.tensor.matmul(
                        half, kvt8_t(cp, t), wv_v(cp),
                        start=(cp == 0), stop=(cp == CP - 1), perf_mode=DR)
                if t % 2 == 1:
                    kcp = t // 2
                    va4 = vaug_sb[kcp][:].rearrange("p (h i c) -> p h i c", h=H, i=2)
                    dst = va4[:, :, :, 0:HD].transpose((0, 2, 1, 3))
                    srcv = ps[:].rearrange("p (i h c) -> p i h c", i=2, h=H)
                    if kcp % 2 == 0:
                        nc.scalar.copy(dst, srcv)
                    else:
                        nc.vector.tensor_copy(dst, srcv)

            def vaug_ones(kcp):
                va4 = vaug_sb[kcp][:].rearrange("p (h i c) -> p h i c", h=H, i=2)
                nc.gpsimd.memset(va4[:, :, :, HD:HD + 1], 1.0)

            # ---- attention ----
            # head h lives on partitions 64*(h%2)..+64 of chunk m=h//2 (no
            # permutation needed). Plain fp8 (no DR): stationary [64,128] gets
            # FWL-fast LDWEIGHTS; the two heads of a pair occupy disjoint row
            # halves -> both matmuls run concurrently, full-array utilization.
            def scores(hp, kc):
                g = hp // 2
                mb = hp % 2
                psc = scps.tile([128, 1024], F32, name="psc", tag="sc")
                for j in range(2):
                    b = 64 * j
                    nc.tensor.matmul(
                        psc[:, j * 512:(j + 1) * 512],
                        kt_sb[g][b:b + 64, mb * 2048 + kc * 128: mb * 2048 + (kc + 1) * 128],
                        qt_sb[g][b:b + 64, mb * 512:(mb + 1) * 512],
                        start=True, stop=True,
                        tile_position=(b, 0))
                return psc

            def exp_op(hp, kc, psc, pt, use_dve):
                parity = kc % 2
                pview = pt[:].rearrange("p (j i q) -> p j i q", j=2, i=2)[:, :, parity, :]
                sview = psc[:].rearrange("p (j q) -> p j q", j=2)
                if use_dve:
                    # uint8 convert saturates negatives to 0 -> prob +0.0
                    nc.vector.tensor_scalar(
                        out=pview.bitcast(U8), in0=sview, scalar1=float(A8),
                        scalar2=float(B8), op0=ALU.mult, op1=ALU.add)
                else:
                    nc.scalar.activation(pview, sview, AF.Exp,
                                         bias=ebias[:], scale=float(SM_SCALE))

            def pv(hp, kcp, ppv, pt):
                for j in range(2):
                    h = 2 * hp + j
                    va4 = vaug_sb[kcp][:].rearrange("p (h i c) -> p h i c", h=H, i=2)
                    nc.tensor.matmul(
                        ppv[j][:],
                        va4[:, h, :, 0:HD + 1],
                        pt[:].rearrange("p (j i q) -> p j i q", j=2, i=2)[:, j],
                        start=(kcp == 0), stop=(kcp == KCP - 1), perf_mode=DR,
                        skip_group_check=True)

            resid_sb, var_sb = {}, {}

            def finish(hp, then_ln=False):
                cts = []
                ptrs = {}
                for j in range(2):
                    ct = wkp.tile([HD + 1, QBLK], F32, name=f"ct{j}", tag=f"ct{j}")
                    if j == 0 or then_ln:
                        nc.scalar.copy(ct[:], ppvs[hp][j][:])
                    else:
                        nc.vector.tensor_copy(ct[:], ppvs[hp][j][:])
                    cts.append(ct)
                for j in range(2):
                    h = 2 * hp + j
                    ptr = scps.tile([128, 4 * 66], F32, name="ptr", tag="sc")
                    for q in range(4):
                        nc.tensor.transpose(
                            ptr[:, q * 66:q * 66 + 65],
                            cts[j][:, q * 128:(q + 1) * 128],
                            ident[0:HD + 1, 0:HD + 1])
                    if not then_ln:
                        linv4 = smp.tile([128, 4], F32, name="linv", tag="linv")
                        nc.vector.reciprocal(
                            linv4[:], ptr[:].rearrange("p (q c) -> p q c", q=4)[:, :, HD])
                        cview = cbig[:].rearrange("p (q d) -> p q d", q=4)[:, :, h * HD:(h + 1) * HD]
                        pview = ptr[:].rearrange("p (q c) -> p q c", q=4)[:, :, 0:HD]
                        lbc = linv4[:].unsqueeze(2).to_broadcast((128, 4, HD))
                        nc.vector.tensor_tensor(
                            out=cview, in0=pview, in1=lbc, op=ALU.mult)
                    else:
                        ptrs[j] = ptr
                if then_ln:
                    # last head pair: per-q pipeline so LN/output overlap
                    for q in range(4):
                        for j in range(2):
                            h = 2 * hp + j
                            lv = smp.tile([128, 1], F32, name="lv", tag=f"lv{q}{j}")
                            nc.vector.reciprocal(
                                lv[:], ptrs[j][:, q * 66 + HD:q * 66 + HD + 1])
                            # ACT does the normalize so DVE is free for LN
                            nc.scalar.activation(
                                cbig[:, q * 512 + h * HD:q * 512 + (h + 1) * HD],
                                ptrs[j][:, q * 66:q * 66 + HD],
                                AF.Copy, bias=0.0, scale=lv[:])
                        ln_head(q)
                        ln_tail_q(q)

            def ln_head(q):
                resid = pp.tile([128, 512], F32, name=f"res{q}", tag=f"res{q}")
                rowsum = smp.tile([128, 1], F32, name="rs", tag=f"rs{q}")
                nc.vector.scalar_tensor_tensor(
                    out=resid[:], in0=cbig[:, q * 512:(q + 1) * 512], scalar=0.0,
                    in1=xqb[:, q * 512:(q + 1) * 512],
                    op0=ALU.bypass, op1=ALU.add, accum_out=rowsum[:])
                sq = wkp.tile([128, 512], F32, name="sq", tag="sq")
                sqs = smp.tile([128, 1], F32, name="sqs", tag=f"sqs{q}")
                nc.scalar.activation(sq[:], resid[:], AF.Square, accum_out=sqs[:])
                mu = smp.tile([128, 1], F32, name="mu", tag=f"mu{q}")
                nc.vector.tensor_scalar_mul(mu[:], rowsum[:], 1.0 / D)
                musq = smp.tile([128, 1], F32, name="musq", tag=f"musq{q}")
                nc.vector.tensor_tensor(out=musq[:], in0=mu[:], in1=mu[:], op=ALU.mult)
                var = smp.tile([128, 1], F32, name="var", tag=f"var{q}")
                nc.vector.scalar_tensor_tensor(
                    out=var[:], in0=sqs[:], scalar=1.0 / D, in1=musq[:],
                    op0=ALU.mult, op1=ALU.subtract)
                var_sb[q] = var
                # tA = (resid - mu) * gamma  (gamma broadcast tile)
                tA = pp.tile([128, 512], F32, name=f"tA{q}", tag=f"tA{q}")
                nc.vector.scalar_tensor_tensor(
                    out=tA[:], in0=resid[:], scalar=mu[:], in1=gamma,
                    op0=ALU.subtract, op1=ALU.mult)
                resid_sb[q] = tA

            def ln_tail_q(q):
                std = smp.tile([128, 1], F32, name="std", tag=f"std{q}")
                nc.scalar.activation(std[:], var_sb[q][:], AF.Sqrt, bias=eps_sb[:])
                inv = smp.tile([128, 1], F32, name="inv", tag=f"inv{q}")
                nc.vector.reciprocal(inv[:], std[:])
                o = wkp.tile([128, 512], F16, name="o", tag="o")
                nc.vector.scalar_tensor_tensor(
                    out=o[:], in0=resid_sb[q][:], scalar=inv[:], in1=beta,
                    op0=ALU.mult, op1=ALU.add)
                nc.sync.dma_start(out[q * 128:(q + 1) * 128, :], o[:])

            # ---- schedule ----
            proj_qt(0)
            warm_burst(8)
            proj_kt(0, 0)
            proj_qt(1)
            proj_kt(1, 0)
            for t in range(4):
                proj_v(t)
                if t % 2 == 1:
                    vaug_ones(t // 2)
            proj_kt(0, 1)
            proj_kt(1, 1)
            for t in range(4, 10):
                proj_v(t)
                if t % 2 == 1:
                    vaug_ones(t // 2)
            proj_qt(2)
            proj_qt(3)
            late_work = [("v", 10), ("v", 11),
                         ("v", 12), ("v", 13), ("v", 14), ("v", 15),
                         None, None, None, None, None, None, None, None,
                         None, None,
                         ("kt", 2, 0), None, ("kt", 3, 0), None,
                         ("kt", 2, 1), None, ("kt", 3, 1)]
            ppvs = {}
            pending = None
            for hp in range(HP):
                ppv = [pvps.tile([HD + 1, QBLK], F32, name=f"pv{j}", tag=f"pv{j}")
                       for j in range(2)]
                ppvs[hp] = ppv
                for kc in range(KC):
                    if late_work:
                        w = late_work.pop(0)
                        if w is None:
                            pass
                        elif w[0] == "kt":
                            proj_kt(w[1], w[2])
                        elif w[0] == "qt":
                            proj_qt(w[1])
                        else:
                            t = w[1]
                            proj_v(t)
                            if t % 2 == 1:
                                vaug_ones(t // 2)
                    psc = scores(hp, kc)
                    kcp = kc // 2
                    unit = hp * 8 + kcp
                    if kc % 2 == 0:
                        tag = "pta" if unit % 2 == 0 else "ptb"
                        pt_cur = wkp.tile([128, 2048], F8E5, name=tag, tag=tag)
                    use_dve = (kc % 2 == 1) and unit not in BOTH_ACT_UNITS
                    exp_op(hp, kc, psc, pt_cur, use_dve)
                    if kc % 2 == 1:
                        pv(hp, kcp, ppv, pt_cur)
                if pending is not None:
                    finish(pending)
                pending = hp
            finish(pending, then_ln=True)

    nc.compile()
    return nc


_PROGRAM = None


def _get_program():
    global _PROGRAM
    if _PROGRAM is None:
        _PROGRAM = _build_program()
    return _PROGRAM


def _dr_pack(m):
    """[512, C] contraction-major -> per chunk-pair [128, 2*C] fp8 tiles."""
    res = []
    for cp in range(CP):
        t = m.reshape(2, 2, 128, m.shape[1])[cp].transpose(1, 0, 2).reshape(128, -1)
        res.append(np.ascontiguousarray(t, dtype=NP8))
    return res


def _chunks_bf(m):
    """[512, C] -> [128, 4*C] bf16: 4 row-chunks side by side."""
    return np.ascontiguousarray(
        m.reshape(4, 128, m.shape[1]).transpose(1, 0, 2).reshape(128, -1), NPBF)


def _wblocks_mm(w):
    """[512, 512] -> [128, 2048] bf16, m-major: [p, m*512 + c*128 + r] =
    w[c*128 + p, m*128 + r]."""
    return np.ascontiguousarray(
        w.reshape(4, 128, 4, 128).transpose(1, 2, 0, 3).reshape(128, 2048), NPBF)


def _make_in_maps(query, key_value, Wq, bq, Wk, bk, Wv, bv, ln_gamma, ln_beta):
    f16, f32 = np.float16, np.float32
    ta_w = _wblocks_mm(Wq)                       # [128, 2048] m-major
    ta2 = _wblocks_mm(Wk)                        # [128, 2048] m-major
    wv8 = _dr_pack(Wv)
    te = np.ascontiguousarray(bq.reshape(4, 128).T, f32)
    gb = np.empty((128, 1024), f32)
    gb[:, 0:512] = ln_gamma[None, :]
    gb[:, 512:1024] = ln_beta[None, :]

    tbs, tcs = [], []
    for b in range(B):
        kvT = np.ascontiguousarray(key_value[b].T)          # [512, 2048]
        kb = _chunks_bf(kvT)                                # [128, 4*2048]
        kb4 = kb.reshape(128, 4, 2048)
        tb0 = np.ascontiguousarray(kb4[:, :, 0:1024].reshape(128, 4096))
        tb1 = np.ascontiguousarray(kb4[:, :, 1024:2048].reshape(128, 4096))
        kvt8 = _dr_pack(kvT)
        tc_ = np.empty((128, 10240), NP8)
        for kq in range(4):
            for cp in range(CP):
                blk = kvt8[cp].reshape(128, 2, 2048)[:, :, kq * 512:(kq + 1) * 512]
                tc_[:, kq * 2048 + cp * 1024:kq * 2048 + (cp + 1) * 1024] = \
                    blk.reshape(128, 1024)
        tc_[:, 8192:9216] = wv8[0]
        tc_[:, 9216:10240] = wv8[1]
        tbs.append((tb0, tb1))
        tcs.append(tc_)

    in_maps = []
    for core in range(N_CORES):
        b, qb = divmod(core, 4)
        blk = query[b, qb * QBLK:(qb + 1) * QBLK, :]
        ta = np.empty((128, 4096), NPBF)
        ta[:, 0:2048] = _chunks_bf(np.ascontiguousarray(blk.T))
        ta[:, 2048:4096] = ta_w
        xqb = blk + bv[None, :]
        td = np.empty((128, 3072), f16)
        td[:, 0:2048] = xqb.reshape(4, 128, 512).transpose(1, 0, 2).reshape(128, 2048)
        td[:, 2048:3072] = gb
        m = {"ta": ta, "ta2": ta2, "tb0": tbs[b][0], "tb1": tbs[b][1],
             "tc": tcs[b], "td": td, "te": te}
        in_maps.append(m)
    return in_maps


def kernel(query, key_value, Wq, bq, Wk, bk, Wv, bv, ln_gamma, ln_beta,
           _trace=False, _trace_kwargs=None):
    args = [np.asarray(a, dtype=np.float32) for a in
            (query, key_value, Wq, bq, Wk, bk, Wv, bv, ln_gamma, ln_beta)]
    nc = _get_program()
    in_maps = _make_in_maps(*args)
    res = bass_utils.run_bass_kernel_spmd(
        nc, in_maps, core_ids=list(range(N_CORES)), trace=_trace,
        **(_trace_kwargs or {}))
    out = np.empty((B, N_Q, D), np.float32)
    for core in range(N_CORES):
        b, qb = divmod(core, 4)
        out[b, qb * QBLK:(qb + 1) * QBLK, :] = res.results[core]["out"].astype(np.float32)
    if _trace:
        return out, res
    return out


# revision 45
# speedup vs baseline: 1.0013x; 1.0013x over previous
"""Trainium2 Bass kernel for CrossModalAttention (attention + residual + LayerNorm).

Math: the reference concatenates [query, key_value], projects Q/K/V, attends with an
additive -10000 mask on key columns < N_q, and keeps only the query-row outputs.
exp(-10000 + s) == 0 in fp32, so this is exactly standard cross-attention:
    Q = query @ Wq + bq ; K = key_value @ Wk (+bk) ; V = key_value @ Wv + bv
    C = softmax(Q K^T / 8) V ;  out = LayerNorm(query + C) * gamma + beta
bk shifts every score of a query row equally (score += Q_q . bk for all keys),
so softmax cancels it -> bk is dropped exactly. bv is constant over keys, so
ctx = PV/denom + bv -> folded into the residual (xqb = query + bv) host-side.

Sharding: 8 cores = 2 batches x 4 query-blocks of 512 rows. Each core computes
K/V projections for its batch (duplicated across the 4 cores of a batch) and
full 8-head attention + LN for its query block.

Per-core pipeline (v3):
  - Q/K projections in bf16 (fp8 upstream puts max-rel-err at the 2e-2 gate;
    fp8 K alone measures 2.9e-2), outputs stored e4m3. V path fully fp8-DR.
  - Head h's 64 dims land naturally on partitions 64*(h%2)..+64 of chunk
    h//2 (no permutation). Scores run plain fp8 (not DoubleRow: the 256-col
    DR stationary load is slower than the matmul it feeds): [64,128]
    stationaries get FWL-fast LDWEIGHTS and the two heads of a pair occupy
    disjoint row halves -> concurrent matmuls, full-array utilization.
  - probs e5m2 everywhere (e4m3 max 240 overflows to inf on observed
    scores): even kc -> ScalarE ACT exp, odd kc -> VectorE Schraudolph
    affine bit-cast through uint8 (saturates negatives to +0; no NaN
    cliffs). Per-kc engine split makes the two exps of each psc pair run
    concurrently; a few units send both halves to ACT for balance.
  - PV in fp8 DR over ones-augmented V (row 64 = softmax denominator);
    PE-transpose + reciprocal + broadcast-stride multiply normalize; the
    final head pair is processed per-q-chunk so LayerNorm and output DMA
    overlap the finish.
  - 3-deep [128,1024] PSUM scoring pool (6 banks) + 2 PV banks; DMAs are
    batched into 7 tensors across 3 rings, m-major weight blocks so each
    projection step waits only on its own slice.
"""

import os
import sys

import numpy as np

try:
    import concourse.bass as bass  # noqa: F401
except ImportError:
    for _p in ("/opt/trn_rl_repo", "/root/.axon_site/_ro/trn_rl_repo"):
        if os.path.isdir(_p):
            sys.path.insert(0, _p)
            break
    import concourse.bass as bass  # noqa: F401

import ml_dtypes
import concourse.tile as tile
from concourse import bacc, bass_utils, mybir
from concourse.masks import make_identity

F32 = mybir.dt.float32
F16 = mybir.dt.float16
BF16 = mybir.dt.bfloat16
F8 = mybir.dt.float8e4
F8E5 = mybir.dt.float8e5
U8 = mybir.dt.uint8
AF = mybir.ActivationFunctionType
ALU = mybir.AluOpType
DR = mybir.MatmulPerfMode.DoubleRow
NP8 = mybir.dt.np(F8)
NPBF = ml_dtypes.bfloat16

B, N_Q, N_KV, D, H, HD = 2, 2048, 2048, 512, 8, 64
N_CORES = 8
QBLK = N_Q // 4          # 512 query rows per core
DC = D // 128            # 4 partition chunks of the model dim
CP = DC // 2             # 2 chunk-pairs (fp8 DoubleRow)
KC = N_KV // 128         # 16 key chunks
KCP = KC // 2            # 8 key chunk-pairs
HP = H // 2              # 4 head pairs
LN_EPS = 1e-5
SM_SCALE = 0.125
EXP_BIAS = -2.0          # probs = exp(s/8 - 2); e5m2 has no reachable cliff

# Schraudolph constants for e5m2 output via uint8: u8 = A8*s_raw + B8
A8 = 4.0 * np.log2(np.e) * SM_SCALE
B8 = 60.0 + 4.0 * np.log2(np.e) * EXP_BIAS - 4.0 * np.log2(1.0301)

# exp engine per (unit, kc): even kc -> ScalarE ACT exp, odd kc -> VectorE
# Schraudolph, so the two exps of a unit run concurrently on both engines.
# A few units send both halves to ACT to balance total throughput (36/28).
BOTH_ACT_UNITS = {11, 27}


def _build_program():
    nc = bacc.Bacc("TRN2", target_bir_lowering=False, debug=False,
                   num_devices=N_CORES)

    def din(name, shape, dt):
        return nc.dram_tensor(name, shape, dt, kind="ExternalInput").ap()

    # batched input tensors (few, large DMAs)
    ta = din("ta", [128, 4096], BF16)    # xqt c0..3 (2048) | wq c0..3 (2048)
    ta2 = din("ta2", [128, 2048], BF16)  # wk c0..3
    tb = [din(f"tb{h}", [128, 4096], BF16) for h in range(2)]  # kvT [c(4) x 1024keys]
    tc_ = din("tc", [128, 10240], F8)    # kvt8 key-quarters (8192) | wv8 (2048)
    td = din("td", [128, 3072], F16)     # xqb (2048) | gamma (512) | beta (512)
    te = din("te", [128, 4], F32)        # bq chunked
    out = nc.dram_tensor("out", [QBLK, D], F16, kind="ExternalOutput").ap()

    with tile.TileContext(nc) as tc:
        with (
            tc.tile_pool(name="persist", bufs=1) as pp,
            tc.tile_pool(name="work", bufs=4) as wkp,
            tc.tile_pool(name="small", bufs=8) as smp,
            tc.tile_pool(name="sc_ps", bufs=3, space="PSUM") as scps,
            tc.tile_pool(name="pv_ps", bufs=1, space="PSUM") as pvps,
        ):
            # ---- persistent tiles + loads ----
            a_sb = pp.tile([128, 4096], BF16, name="ta", tag="ta")
            a2_sb = pp.tile([128, 2048], BF16, name="ta2", tag="ta2")
            b_sb = [pp.tile([128, 4096], BF16, name=f"tb{h}", tag=f"tb{h}")
                    for h in range(2)]
            c_sb = pp.tile([128, 10240], F8, name="tc", tag="tc")
            d_sb = pp.tile([128, 3072], F16, name="td", tag="td")
            e_sb = pp.tile([128, 4], F32, name="te", tag="te")

            # four DMA rings; W blocks are m-major and the fp8 tensor is
            # key-quarter-major so every proj step waits only on its slice.
            nc.sync.dma_start(e_sb[:], te)
            nc.sync.dma_start(a_sb[:, 0:2560], ta[:, 0:2560])     # xqt + wq m0
            nc.sync.dma_start(a2_sb[:, 0:512], ta2[:, 0:512])     # wk m0
            nc.sync.dma_start(b_sb[0][:, 0:2048], tb[0][:, 0:2048])
            nc.sync.dma_start(a_sb[:, 2560:4096], ta[:, 2560:4096])
            nc.sync.dma_start(a2_sb[:, 512:1024], ta2[:, 512:1024])
            nc.gpsimd.dma_start(c_sb[:, 8192:10240], tc_[:, 8192:10240])  # wv
            nc.gpsimd.dma_start(c_sb[:, 0:2048], tc_[:, 0:2048])          # keys q0
            nc.gpsimd.dma_start(c_sb[:, 2048:4096], tc_[:, 2048:4096])    # keys q1
            nc.gpsimd.dma_start(c_sb[:, 4096:8192], tc_[:, 4096:8192])    # keys q2,q3
            nc.scalar.dma_start(b_sb[0][:, 2048:4096], tb[0][:, 2048:4096])
            nc.scalar.dma_start(d_sb[:], td)
            nc.scalar.dma_start(b_sb[1][:], tb[1])
            nc.scalar.dma_start(a2_sb[:, 1024:2048], ta2[:, 1024:2048])

            def xqt_c(c):
                return a_sb[:, c * 512:(c + 1) * 512]

            def wq_cm(c, m):
                return a_sb[:, 2048 + m * 512 + c * 128: 2048 + m * 512 + (c + 1) * 128]

            def wk_cm(c, m):
                return a2_sb[:, m * 512 + c * 128: m * 512 + (c + 1) * 128]

            def kvt8_t(cp, t):
                # key-quarter kq = t//4 block: [kq][cp][i][512 keys]
                kq, tr = t // 4, t % 4
                base = kq * 2048 + cp * 1024
                return c_sb[:, base:base + 1024].rearrange(
                    "p (i n) -> p i n", i=2)[:, :, tr * 128:(tr + 1) * 128]

            def wv_v(cp):
                return c_sb[:, 8192 + cp * 1024: 8192 + (cp + 1) * 1024].rearrange(
                    "p (i n) -> p i n", i=2)

            xqb = d_sb[:, 0:2048]
            gamma = d_sb[:, 2048:2560]
            beta = d_sb[:, 2560:3072]

            # HAM warmup: ~7us of back-to-back dummy matmuls during the
            # input-DMA shadow gets the PE to 2.4GHz before the real
            # projections start (otherwise they all run at the cold 1.2GHz).
            wrm = pp.tile([128, 512], BF16, name="wrm", tag="wrm")
            nc.vector.memset(wrm[:], 0.125)

            def warm_burst(n):
                for _i in range(n):
                    psd = scps.tile([128, 512], F32, name="psd", tag="sc")
                    nc.tensor.matmul(psd[:], wrm[:, 0:128], wrm[:],
                                     start=True, stop=True)
            warm_burst(28)

            ident = pp.tile([128, 128], F32, name="ident", tag="ident")
            make_identity(nc, ident[:])
            ebias = pp.tile([128, 1], F32, name="ebias", tag="ebias")
            nc.vector.memset(ebias[:], float(EXP_BIAS))
            eps_sb = pp.tile([128, 1], F32, name="eps", tag="eps")
            nc.vector.memset(eps_sb[:], float(LN_EPS))

            qt_sb = [pp.tile([128, 1024], F8, name=f"qt{g}", tag=f"qt{g}") for g in range(2)]
            kt_sb = [pp.tile([128, 4096], F8, name=f"kt{g}", tag=f"kt{g}") for g in range(2)]
            vaug_sb = [pp.tile([128, 2048], F8, name=f"va{t}", tag=f"va{t}")
                       for t in range(KCP)]
            cbig = pp.tile([128, 2048], F32, name="cbig", tag="cbig")

            def proj_qt(m):
                ps = scps.tile([128, 512], F32, name="ps_qt", tag="sc")
                for c in range(DC):
                    nc.tensor.matmul(ps[:], wq_cm(c, m), xqt_c(c),
                                     start=(c == 0), stop=(c == DC - 1))
                nc.scalar.add(
                    qt_sb[m // 2][:, (m % 2) * 512:(m % 2 + 1) * 512],
                    ps[:], e_sb[:, m:m + 1])

            def proj_kt(m, nbb):
                # keys nbb*1024 .. +1024 of output-dim chunk m (bf16, no bias)
                ps = scps.tile([128, 1024], F32, name="ps_kt", tag="sc")
                for nn in range(2):
                    for c in range(DC):
                        mov = b_sb[nbb][:, c * 1024 + nn * 512: c * 1024 + (nn + 1) * 512]
                        nc.tensor.matmul(ps[:, nn * 512:(nn + 1) * 512],
                                         wk_cm(c, m), mov,
                                         start=(c == 0), stop=(c == DC - 1))
                dst = kt_sb[m // 2][:, (m % 2) * 2048 + nbb * 1024:(m % 2) * 2048 + (nbb + 1) * 1024]
                if m < 2:
                    nc.vector.tensor_copy(dst, ps[:])
                else:
                    nc.scalar.copy(dst, ps[:])

            _vps = {}

            def proj_v(t):
                # pair t,t+1 into one [128,1024] psum; one copy per pair
                if t % 2 == 0:
                    _vps[t // 2] = scps.tile([128, 1024], F32, name="ps_v", tag="sc")
                ps = _vps[t // 2]
                half = ps[:, (t % 2) * 512:(t % 2 + 1) * 512]
                for cp in range(CP):
                    nc.tensor.matmul(
                        half, kvt8_t(cp, t), wv_v(cp),
                        start=(cp == 0), stop=(cp == CP - 1), perf_mode=DR)
                if t % 2 == 1:
                    kcp = t // 2
                    va4 = vaug_sb[kcp][:].rearrange("p (h i c) -> p h i c", h=H, i=2)
                    dst = va4[:, :, :, 0:HD].transpose((0, 2, 1, 3))
                    srcv = ps[:].rearrange("p (i h c) -> p i h c", i=2, h=H)
                    if kcp % 2 == 0:
                        nc.scalar.copy(dst, srcv)
                    else:
                        nc.vector.tensor_copy(dst, srcv)

            def vaug_ones(kcp):
                va4 = vaug_sb[kcp][:].rearrange("p (h i c) -> p h i c", h=H, i=2)
                nc.gpsimd.memset(va4[:, :, :, HD:HD + 1], 1.0)

            # ---- attention ----
            # head h lives on partitions 64*(h%2)..+64 of chunk m=h//2 (no
            # permutation needed). Plain fp8 (no DR): stationary [64,128] gets
            # FWL-fast LDWEIGHTS; the two heads of a pair occupy disjoint row
            # halves -> both matmuls run concurrently, full-array utilization.
            def scores(hp, kc):
                g = hp // 2
                mb = hp % 2
                psc = scps.tile([128, 1024], F32, name="psc", tag="sc")
                for j in range(2):
                    b = 64 * j
                    nc.tensor.matmul(
                        psc[:, j * 512:(j + 1) * 512],
                        kt_sb[g][b:b + 64, mb * 2048 + kc * 128: mb * 2048 + (kc + 1) * 128],
                        qt_sb[g][b:b + 64, mb * 512:(mb + 1) * 512],
                        start=True, stop=True,
                        tile_position=(b, 0))
                return psc

            def exp_op(hp, kc, psc, pt, use_dve):
                parity = kc % 2
                pview = pt[:].rearrange("p (j i q) -> p j i q", j=2, i=2)[:, :, parity, :]
                sview = psc[:].rearrange("p (j q) -> p j q", j=2)
                if use_dve:
                    # uint8 convert saturates negatives to 0 -> prob +0.0
                    nc.vector.tensor_scalar(
                        out=pview.bitcast(U8), in0=sview, scalar1=float(A8),
                        scalar2=float(B8), op0=ALU.mult, op1=ALU.add)
                else:
                    nc.scalar.activation(pview, sview, AF.Exp,
                                         bias=ebias[:], scale=float(SM_SCALE))

            def pv(hp, kcp, ppv, pt):
                for j in range(2):
                    h = 2 * hp + j
                    va4 = vaug_sb[kcp][:].rearrange("p (h i c) -> p h i c", h=H, i=2)
                    nc.tensor.matmul(
                        ppv[j][:],
                        va4[:, h, :, 0:HD + 1],
                        pt[:].rearrange("p (j i q) -> p j i q", j=2, i=2)[:, j],
                        start=(kcp == 0), stop=(kcp == KCP - 1), perf_mode=DR,
                        skip_group_check=True)

            resid_sb, var_sb = {}, {}

            def finish(hp, then_ln=False):
                cts = []
                ptrs = {}
                for j in range(2):
                    ct = wkp.tile([HD + 1, QBLK], F32, name=f"ct{j}", tag=f"ct{j}")
                    if j == 0:
                        nc.scalar.copy(ct[:], ppvs[hp][j][:])
                    else:
                        nc.vector.tensor_copy(ct[:], ppvs[hp][j][:])
                    cts.append(ct)
                for j in range(2):
                    h = 2 * hp + j
                    ptr = scps.tile([128, 4 * 66], F32, name="ptr", tag="sc")
                    for q in range(4):
                        nc.tensor.transpose(
                            ptr[:, q * 66:q * 66 + 65],
                            cts[j][:, q * 128:(q + 1) * 128],
                            ident[0:HD + 1, 0:HD + 1])
                    if not then_ln:
                        linv4 = smp.tile([128, 4], F32, name="linv", tag="linv")
                        nc.vector.reciprocal(
                            linv4[:], ptr[:].rearrange("p (q c) -> p q c", q=4)[:, :, HD])
                        cview = cbig[:].rearrange("p (q d) -> p q d", q=4)[:, :, h * HD:(h + 1) * HD]
                        pview = ptr[:].rearrange("p (q c) -> p q c", q=4)[:, :, 0:HD]
                        lbc = linv4[:].unsqueeze(2).to_broadcast((128, 4, HD))
                        nc.vector.tensor_tensor(
                            out=cview, in0=pview, in1=lbc, op=ALU.mult)
                    else:
                        ptrs[j] = ptr
                if then_ln:
                    # last head pair: per-q pipeline so LN/output overlap
                    for q in range(4):
                        for j in range(2):
                            h = 2 * hp + j
                            lv = smp.tile([128, 1], F32, name="lv", tag=f"lv{q}{j}")
                            nc.vector.reciprocal(
                                lv[:], ptrs[j][:, q * 66 + HD:q * 66 + HD + 1])
                            nc.vector.tensor_scalar(
                                out=cbig[:, q * 512 + h * HD:q * 512 + (h + 1) * HD],
                                in0=ptrs[j][:, q * 66:q * 66 + HD],
                                scalar1=lv[:], scalar2=None, op0=ALU.mult)
                        ln_head(q)
                        ln_tail_q(q)

            def ln_head(q):
                resid = pp.tile([128, 512], F32, name=f"res{q}", tag=f"res{q}")
                rowsum = smp.tile([128, 1], F32, name="rs", tag=f"rs{q}")
                nc.vector.scalar_tensor_tensor(
                    out=resid[:], in0=cbig[:, q * 512:(q + 1) * 512], scalar=0.0,
                    in1=xqb[:, q * 512:(q + 1) * 512],
                    op0=ALU.bypass, op1=ALU.add, accum_out=rowsum[:])
                sq = wkp.tile([128, 512], F32, name="sq", tag="sq")
                sqs = smp.tile([128, 1], F32, name="sqs", tag=f"sqs{q}")
                nc.scalar.activation(sq[:], resid[:], AF.Square, accum_out=sqs[:])
                mu = smp.tile([128, 1], F32, name="mu", tag=f"mu{q}")
                nc.vector.tensor_scalar_mul(mu[:], rowsum[:], 1.0 / D)
                musq = smp.tile([128, 1], F32, name="musq", tag=f"musq{q}")
                nc.vector.tensor_tensor(out=musq[:], in0=mu[:], in1=mu[:], op=ALU.mult)
                var = smp.tile([128, 1], F32, name="var", tag=f"var{q}")
                nc.vector.scalar_tensor_tensor(
                    out=var[:], in0=sqs[:], scalar=1.0 / D, in1=musq[:],
                    op0=ALU.mult, op1=ALU.subtract)
                var_sb[q] = var
                # tA = (resid - mu) * gamma  (gamma broadcast tile)
                tA = pp.tile([128, 512], F32, name=f"tA{q}", tag=f"tA{q}")
                nc.vector.scalar_tensor_tensor(
                    out=tA[:], in0=resid[:], scalar=mu[:], in1=gamma,
                    op0=ALU.subtract, op1=ALU.mult)
                resid_sb[q] = tA

            def ln_tail_q(q):
                std = smp.tile([128, 1], F32, name="std", tag=f"std{q}")
                nc.scalar.activation(std[:], var_sb[q][:], AF.Sqrt, bias=eps_sb[:])
                inv = smp.tile([128, 1], F32, name="inv", tag=f"inv{q}")
                nc.vector.reciprocal(inv[:], std[:])
                o = wkp.tile([128, 512], F16, name="o", tag="o")
                nc.vector.scalar_tensor_tensor(
                    out=o[:], in0=resid_sb[q][:], scalar=inv[:], in1=beta,
                    op0=ALU.mult, op1=ALU.add)
                nc.sync.dma_start(out[q * 128:(q + 1) * 128, :], o[:])

            # ---- schedule ----
            proj_qt(0)
            warm_burst(8)
            proj_kt(0, 0)
            proj_qt(1)
            proj_kt(1, 0)
            for t in range(4):
                proj_v(t)
                if t % 2 == 1:
                    vaug_ones(t // 2)
            proj_kt(0, 1)
            proj_kt(1, 1)
            for t in range(4, 10):
                proj_v(t)
                if t % 2 == 1:
                    vaug_ones(t // 2)
            proj_qt(2)
            proj_qt(3)
            late_work = [("v", 10), ("v", 11),
                         ("v", 12), ("v", 13), ("v", 14), ("v", 15),
                         None, None, None, None, None, None, None, None,
                         None, None,
                         ("kt", 2, 0), None, ("kt", 3, 0), None,
                         ("kt", 2, 1), None, ("kt", 3, 1)]
            ppvs = {}
            pending = None
            for hp in range(HP):
                ppv = [pvps.tile([HD + 1, QBLK], F32, name=f"pv{j}", tag=f"pv{j}")
                       for j in range(2)]
                ppvs[hp] = ppv
                for kc in range(KC):
                    if late_work:
                        w = late_work.pop(0)
                        if w is None:
                            pass
                        elif w[0] == "kt":
                            proj_kt(w[1], w[2])
                        elif w[0] == "qt":
                            proj_qt(w[1])
                        else:
                            t = w[1]
                            proj_v(t)
                            if t % 2 == 1:
                                vaug_ones(t // 2)
                    psc = scores(hp, kc)
                    kcp = kc // 2
                    unit = hp * 8 + kcp
                    if kc % 2 == 0:
                        tag = "pta" if unit % 2 == 0 else "ptb"
                        pt_cur = wkp.tile([128, 2048], F8E5, name=tag, tag=tag)
                    use_dve = (kc % 2 == 1) and unit not in BOTH_ACT_UNITS
                    exp_op(hp, kc, psc, pt_cur, use_dve)
                    if kc % 2 == 1:
                        pv(hp, kcp, ppv, pt_cur)
                if pending is not None:
                    finish(pending)
                pending = hp
            finish(pending, then_ln=True)

    nc.compile()
    return nc


_PROGRAM = None


def _get_program():
    global _PROGRAM
    if _PROGRAM is None:
        _PROGRAM = _build_program()
    return _PROGRAM


def _dr_pack(m):
    """[512, C] contraction-major -> per chunk-pair [128, 2*C] fp8 tiles."""
    res = []
    for cp in range(CP):
        t = m.reshape(2, 2, 128, m.shape[1])[cp].transpose(1, 0, 2).reshape(128, -1)
        res.append(np.ascontiguousarray(t, dtype=NP8))
    return res


def _chunks_bf(m):
    """[512, C] -> [128, 4*C] bf16: 4 row-chunks side by side."""
    return np.ascontiguousarray(
        m.reshape(4, 128, m.shape[1]).transpose(1, 0, 2).reshape(128, -1), NPBF)


def _wblocks_mm(w):
    """[512, 512] -> [128, 2048] bf16, m-major: [p, m*512 + c*128 + r] =
    w[c*128 + p, m*128 + r]."""
    return np.ascontiguousarray(
        w.reshape(4, 128, 4, 128).transpose(1, 2, 0, 3).reshape(128, 2048), NPBF)


def _make_in_maps(query, key_value, Wq, bq, Wk, bk, Wv, bv, ln_gamma, ln_beta):
    f16, f32 = np.float16, np.float32
    ta_w = _wblocks_mm(Wq)                       # [128, 2048] m-major
    ta2 = _wblocks_mm(Wk)                        # [128, 2048] m-major
    wv8 = _dr_pack(Wv)
    te = np.ascontiguousarray(bq.reshape(4, 128).T, f32)
    gb = np.empty((128, 1024), f32)
    gb[:, 0:512] = ln_gamma[None, :]
    gb[:, 512:1024] = ln_beta[None, :]

    tbs, tcs = [], []
    for b in range(B):
        kvT = np.ascontiguousarray(key_value[b].T)          # [512, 2048]
        kb = _chunks_bf(kvT)                                # [128, 4*2048]
        kb4 = kb.reshape(128, 4, 2048)
        tb0 = np.ascontiguousarray(kb4[:, :, 0:1024].reshape(128, 4096))
        tb1 = np.ascontiguousarray(kb4[:, :, 1024:2048].reshape(128, 4096))
        kvt8 = _dr_pack(kvT)
        tc_ = np.empty((128, 10240), NP8)
        for kq in range(4):
            for cp in range(CP):
                blk = kvt8[cp].reshape(128, 2, 2048)[:, :, kq * 512:(kq + 1) * 512]
                tc_[:, kq * 2048 + cp * 1024:kq * 2048 + (cp + 1) * 1024] = \
                    blk.reshape(128, 1024)
        tc_[:, 8192:9216] = wv8[0]
        tc_[:, 9216:10240] = wv8[1]
        tbs.append((tb0, tb1))
        tcs.append(tc_)

    in_maps = []
    for core in range(N_CORES):
        b, qb = divmod(core, 4)
        blk = query[b, qb * QBLK:(qb + 1) * QBLK, :]
        ta = np.empty((128, 4096), NPBF)
        ta[:, 0:2048] = _chunks_bf(np.ascontiguousarray(blk.T))
        ta[:, 2048:4096] = ta_w
        xqb = blk + bv[None, :]
        td = np.empty((128, 3072), f16)
        td[:, 0:2048] = xqb.reshape(4, 128, 512).transpose(1, 0, 2).reshape(128, 2048)
        td[:, 2048:3072] = gb
        m = {"ta": ta, "ta2": ta2, "tb0": tbs[b][0], "tb1": tbs[b][1],
             "tc": tcs[b], "td": td, "te": te}
        in_maps.append(m)
    return in_maps


def kernel(query, key_value, Wq, bq, Wk, bk, Wv, bv, ln_gamma, ln_beta,
           _trace=False, _trace_kwargs=None):
    args = [np.asarray(a, dtype=np.float32) for a in
            (query, key_value, Wq, bq, Wk, bk, Wv, bv, ln_gamma, ln_beta)]
    nc = _get_program()
    in_maps = _make_in_maps(*args)
    res = bass_utils.run_bass_kernel_spmd(
        nc, in_maps, core_ids=list(range(N_CORES)), trace=_trace,
        **(_trace_kwargs or {}))
    out = np.empty((B, N_Q, D), np.float32)
    for core in range(N_CORES):
        b, qb = divmod(core, 4)
        out[b, qb * QBLK:(qb + 1) * QBLK, :] = res.results[core]["out"].astype(np.float32)
    if _trace:
        return out, res
    return out
